# revision 1
# baseline (speedup 1.0000x reference)
"""Trainium2 Bass kernel for nn_MicroAdder (tiny dense transformer).

Decomposition: every per-element quantity in the reference network is either
 (a) affine in the basis [u_s, w_s, 1] where u = cos(tok_angle), w = sin(tok_angle)
     (computed with one ACT Sin op each), with position-dependent constant
     coefficients -> ONE PE matmul per 128-row block produces all 8 needed
     per-element linear forms (a, q0, q1, e0, e1, y0, y1, r), including the
     causal-softmax attention mixing (folded into the host-precomputed R matrix), or
 (b) a short elementwise chain (2 rsqrt, 2 relu, ~15 two-input ops) on those forms, or
 (c) the final (T,V) logits expansion  out = L0 (x) E0 + L1 (x) E1, done by a second
     PE matmul per block with a block-diagonal constant rhs.

Sharding: pure data parallel over the batch dim across 8 NeuronCores.
"""

import math
import sys

import numpy as np

for _p in ("/opt/trn_rl_repo", "/root/.axon_site/_ro/trn_rl_repo"):
    if _p not in sys.path:
        sys.path.append(_p)

import concourse.bacc as bacc  # noqa: E402
import concourse.bass as bass  # noqa: E402
import concourse.tile as tile  # noqa: E402
from concourse import mybir  # noqa: E402
from concourse.bass_utils import run_bass_kernel_spmd  # noqa: E402
from concourse.masks import make_identity  # noqa: E402

# ---------------------------------------------------------------- problem dims
B, T, V = 65536, 34, 14
D, EPS, MAX_DIGITS = 5, 1e-5, 10
NCORES = 8
BC = B // NCORES            # rows per core = 8192
P = 128                     # partitions
NPER = BC // P              # rows per partition = 64
NBLK = NPER                 # blocks per core = 64 (block j = rows {p*NPER + j})
SGB = 16                    # blocks per supergroup
NSG = NBLK // SGB           # 4 supergroups
K1 = 2 * T + 1              # basis size = 69
NG = 8                      # matmul1 groups
N1 = NG * T                 # 272
N2 = T * V                  # 476
NPRM = 12

F32 = mybir.dt.float32
I32 = mybir.dt.int32
AF = mybir.ActivationFunctionType
ALU = mybir.AluOpType

# group order in matmul1 output columns (g*T..g*T+T)
G_A, G_Q0, G_Q1, G_E0, G_E1, G_Y0, G_Y1, G_R = range(8)

# PRM slots
(P_SCL, P_BSH, P_RAT, P_SQ0, P_C3, P_H00, P_H10, P_H01, P_H11,
 P_EPS, P_ZERO) = range(11)
CODE_BITS = 24  # idx -> quantized reduced-angle code resolution


# ---------------------------------------------------------------- host tables
def host_tables(tok_A, tok_start, tok_stride, sp_amp, sp_phase, sp_slope, sp_offset,
                norm_w, q_w, q_phase, out_A, out_B, fc1_w, fc2_w, head_w):
    f = np.float64
    A = f(tok_A)
    t = np.arange(T, dtype=f)
    th = 2.0 * np.pi * t / MAX_DIGITS + f(sp_phase)
    pos = np.stack([f(sp_amp) * np.cos(th), f(sp_amp) * np.sin(th),
                    f(sp_slope) * t + f(sp_offset)], axis=-1)
    k = pos @ np.asarray(q_w, f).T
    c0, s0 = np.cos(f(q_phase[0])), np.sin(f(q_phase[0]))
    q = k.copy()
    q[:, 0] = c0 * k[:, 0] - s0 * k[:, 1]
    q[:, 1] = s0 * k[:, 0] + c0 * k[:, 1]
    scores = (q @ k.T) / np.sqrt(f(5.0))
    sm = np.where(np.tril(np.ones((T, T), bool)), scores, -np.inf)
    sm = sm - sm.max(-1, keepdims=True)
    e = np.exp(sm)
    attn = e / e.sum(-1, keepdims=True)

    nw = np.asarray(norm_w, f)
    oA = np.asarray(out_A, f)[:, 0]
    oB = np.asarray(out_B, f)[0]
    S_t = A * A + (pos ** 2).sum(-1)
    rms1 = np.sqrt(S_t / D + EPS)

    M0 = attn * (A * nw[0] * oA[0] / rms1)[None, :]
    M1 = attn * (A * nw[1] * oA[1] / rms1)[None, :]
    c_t = attn @ ((pos * (nw[2:] * oA[2:])[None, :]).sum(-1) / rms1)

    g0 = np.asarray(fc2_w, f)[:, 0]
    g1 = np.asarray(fc2_w, f)[:, 1]
    projs = {
        G_Q0: nw * np.asarray(fc1_w, f)[0],
        G_Q1: nw * np.asarray(fc1_w, f)[1],
        G_E0: 2.0 * g0,
        G_E1: 2.0 * g1,
        G_Y0: nw * np.asarray(head_w, f)[0],
        G_Y1: nw * np.asarray(head_w, f)[1],
    }
    R = np.zeros((K1, NG * T), dtype=f)
    dd = np.eye(T, dtype=f)
    for gi in range(NG):
        cols = slice(gi * T, (gi + 1) * T)
        if gi == G_A:
            R[0:T, cols] = M0.T
            R[T:2 * T, cols] = M1.T
            R[2 * T, cols] = c_t
        elif gi == G_R:
            b2 = (oB ** 2).sum()
            R[0:T, cols] = 2 * A * oB[0] * dd + b2 * M0.T
            R[T:2 * T, cols] = 2 * A * oB[1] * dd + b2 * M1.T
            R[2 * T, cols] = 2 * (pos * oB[None, 2:]).sum(-1) + b2 * c_t
        else:
            v = projs[gi]
            bv = (oB * v).sum()
            R[0:T, cols] = A * v[0] * dd + bv * M0.T
            R[T:2 * T, cols] = A * v[1] * dd + bv * M1.T
            R[2 * T, cols] = (pos * v[None, 2:]).sum(-1) + bv * c_t

    # Basis change for the half-angle scheme actually computed on device:
    #   device computes u' = sin^2(phi), w' = sin(phi)*cos(phi) where
    #   ang = 2*phi + pi  ->  cos(ang) = 2u' - 1, sin(ang) = -2w'.
    # Old basis rows: u = cos(ang), w = sin(ang).
    Rn = R.copy()
    Rn[0:T, :] = 2.0 * R[0:T, :]
    Rn[T:2 * T, :] = -2.0 * R[T:2 * T, :]
    Rn[2 * T, :] = R[2 * T, :] - R[0:T, :].sum(axis=0)
    R = Rn

    G00, G01, G11 = (g0 * g0).sum(), (g0 * g1).sum(), (g1 * g1).sum()
    if G00 > 1e-30:
        sq0, rat = np.sqrt(G00), G01 / G00
        c3 = np.sqrt(max(G11 - G01 * G01 / G00, 0.0))
    else:
        sq0, rat, c3 = 0.0, 0.0, np.sqrt(G11)
    hv0 = nw * np.asarray(head_w, f)[0]
    hv1 = nw * np.asarray(head_w, f)[1]
    H = np.array([[(g0 * hv0).sum(), (g0 * hv1).sum()],
                  [(g1 * hv0).sum(), (g1 * hv1).sum()]])

    dvoc = np.arange(V, dtype=f)
    ang = f(tok_start) + dvoc * f(tok_stride)
    E = np.stack([A * np.cos(ang), A * np.sin(ang)], axis=-1)
    RHS2 = np.zeros((2 * T, N2), dtype=f)
    for t_ in range(T):
        RHS2[t_, t_ * V:(t_ + 1) * V] = E[:, 0]
        RHS2[T + t_, t_ * V:(t_ + 1) * V] = E[:, 1]

    # idx -> code LUT: reduced angle, quantized to CODE_BITS
    angv = np.mod(f(tok_start) + np.arange(V, dtype=f) * f(tok_stride), 2 * np.pi)
    CODE = np.clip(np.round(angv * (2 ** CODE_BITS) / (2 * np.pi)),
                   0, 2 ** CODE_BITS - 1).astype(np.int32)
    half_scale = (2 * np.pi / (2 ** CODE_BITS)) / 2.0  # phi = code*hs - pi/2

    SROW = np.tile(S_t, SGB)[None, :]  # [1, 544]
    PRM = np.zeros((1, NPRM), dtype=f)
    PRM[0, P_SCL] = half_scale
    PRM[0, P_BSH] = -np.pi / 2.0
    PRM[0, P_RAT] = rat
    PRM[0, P_SQ0] = sq0
    PRM[0, P_C3] = c3
    PRM[0, P_H00] = H[0, 0]
    PRM[0, P_H10] = H[1, 0]
    PRM[0, P_H01] = H[0, 1]
    PRM[0, P_H11] = H[1, 1]
    PRM[0, P_EPS] = EPS
    PRM[0, P_ZERO] = 0.0
    return (R.astype(np.float32), RHS2.astype(np.float32),
            np.ascontiguousarray(SROW, np.float32).copy(),
            np.ascontiguousarray(PRM, np.float32).copy(), CODE)


# ---------------------------------------------------------------- bass kernel
def build_bass():
    nc = bacc.Bacc("TRN2", target_bir_lowering=False, debug=False)

    idx_d = nc.dram_tensor("idx", [BC, T], I32, kind="ExternalInput").ap()
    r_d = nc.dram_tensor("R", [K1, N1], F32, kind="ExternalInput").ap()
    rhs2_d = nc.dram_tensor("RHS2", [2 * T, N2], F32, kind="ExternalInput").ap()
    srow_d = nc.dram_tensor("SROW", [1, T * SGB], F32, kind="ExternalInput").ap()
    prm_d = nc.dram_tensor("PRM", [1, NPRM], F32, kind="ExternalInput").ap()
    out_d = nc.dram_tensor("out", [BC, N2], F32, kind="ExternalOutput").ap()

    # DRAM views: partition p holds rows p*NPER .. p*NPER+NPER-1
    idx_v = idx_d.rearrange("(p n) t -> p n t", p=P)       # [128, 64, 34]
    out_v = out_d.rearrange("(p n) c -> p n c", p=P)       # [128, 64, 476]

    FW = T * SGB  # 544 columns per supergroup

    with tile.TileContext(nc) as tc:
        with (
            tc.tile_pool(name="const", bufs=1) as cpool,
            tc.tile_pool(name="idxp", bufs=2) as idxp,
            tc.tile_pool(name="idxf", bufs=2) as idxfp,
            tc.tile_pool(name="uw", bufs=2) as uwp,
            tc.tile_pool(name="dt", bufs=2) as dtp,
            tc.tile_pool(name="lt2", bufs=2) as lt2p,
            tc.tile_pool(name="stage", bufs=2) as stp,
            tc.tile_pool(name="mt", bufs=2) as mtp,
            tc.tile_pool(name="outsb", bufs=3) as outp,
            tc.tile_pool(name="ptp", bufs=2, space="PSUM") as ptp,
            tc.tile_pool(name="pmm1", bufs=2, space="PSUM") as pmm1p,
            tc.tile_pool(name="pout", bufs=2, space="PSUM") as poutp,
        ):
            # ---- constants
            ident = cpool.tile([P, P], F32)
            make_identity(nc, ident[:])
            r_sb = cpool.tile([K1, N1], F32)
            nc.sync.dma_start(r_sb[:], r_d)
            rhs2_sb = cpool.tile([2 * T, N2], F32)
            nc.sync.dma_start(rhs2_sb[:], rhs2_d)
            s_sb = cpool.tile([P, FW], F32)
            nc.sync.dma_start(s_sb[:], srow_d.broadcast_to([P, FW]))
            prm_sb = cpool.tile([P, NPRM], F32)
            nc.sync.dma_start(prm_sb[:], prm_d.broadcast_to([P, NPRM]))

            def prm(i):
                return prm_sb[:, i:i + 1]

            for sg in range(NSG):
                j0 = sg * SGB
                # ---------------- phase A: idx -> u,w basis (interleaved 69-stride)
                idx_t = idxp.tile([P, FW], I32, tag="idx")
                nc.sync.dma_start(idx_t[:], idx_v[:, j0:j0 + SGB, :])
                idxf_t = idxfp.tile([P, FW], F32, tag="idxf")
                nc.vector.tensor_copy(idxf_t[:], idx_t[:])
                uw_t = uwp.tile([P, SGB * K1], F32, tag="uw")   # [128, 16*69]
                uw3 = uw_t[:].rearrange("p (j k) -> p j k", k=K1)
                # half-angle scheme: phi = code*hs - pi/2 in [-pi/2, pi/2]
                # sh = sin(phi); chh = cos(phi) = sin(phi + pi/2) (arg in [0, pi])
                # basis: u' = sh^2, w' = sh*chh
                sh_t = idxfp.tile([P, FW], F32, tag="sh")
                nc.scalar.activation(sh_t[:], idxf_t[:], AF.Sin,
                                     bias=prm(P_BSH), scale=prm(P_SCL))
                chh_t = idxfp.tile([P, FW], F32, tag="chh")
                nc.scalar.activation(chh_t[:], idxf_t[:], AF.Sin,
                                     bias=prm(P_ZERO), scale=prm(P_SCL))
                sh3 = sh_t[:].rearrange("p (j t) -> p j t", t=T)
                chh3 = chh_t[:].rearrange("p (j t) -> p j t", t=T)
                nc.scalar.activation(uw3[:, :, 0:T], sh3, AF.Square,
                                     bias=prm(P_ZERO), scale=1.0)
                nc.vector.tensor_mul(uw3[:, :, T:2 * T], sh3, chh3)
                nc.vector.memset(uw3[:, :, 2 * T:K1], 1.0)

                # ---------------- phase B/C: per-block transpose+matmul1, per-pair drains
                ar_t = stp.tile([P, FW], F32, tag="ar")
                rho_t = stp.tile([P, SGB * 68], F32, tag="rho")
                tab_t = stp.tile([P, SGB * 68], F32, tag="tab")
                yr_t = stp.tile([P, SGB * 102], F32, tag="yr")
                rho3 = rho_t[:].rearrange("p (j c) -> p j c", c=68)
                tab3 = tab_t[:].rearrange("p (j c) -> p j c", c=68)
                yr3 = yr_t[:].rearrange("p (j c) -> p j c", c=102)
                ar3 = ar_t[:].rearrange("p (j t) -> p j t", t=T)

                pt1 = None
                for j in range(SGB):
                    c4 = j % 4
                    if c4 == 0:
                        pt1 = ptp.tile([K1, 4 * P], F32, tag="tp")
                    nc.tensor.transpose(pt1[:, c4 * P:(c4 + 1) * P],
                                        uw3[:, j, :], ident[:])
                    if c4 == 3:
                        dt_t = dtp.tile([K1, 4 * P], F32, tag="dt")
                        nc.vector.tensor_copy(dt_t[:], pt1[:])
                        # matmul1 + drains for the two pairs in these 4 blocks
                        for h in range(2):
                            jj = j - 3 + 2 * h      # first block of pair
                            pr = (jj - j0 * 0) // 2  # pair idx within sg: jj is local
                            pm = pmm1p.tile([P, 1024], F32, tag="mm1")
                            pm3 = pm[:].rearrange("p (b c) -> p b c", b=2)
                            for b in range(2):
                                nc.tensor.matmul(
                                    pm3[:, b, 0:N1],
                                    dt_t[:, (2 * h + b) * P:(2 * h + b + 1) * P],
                                    r_sb[:],
                                    start=True, stop=True)

                            def g(gi):
                                return pm3[:, :, gi * T:(gi + 1) * T]

                            pj = jj  # local block index of first-in-pair
                            # drain y0,y1,r (adjacent groups) to SBUF first
                            nc.vector.tensor_copy(
                                yr3[:, pj:pj + 2, :],
                                pm3[:, :, G_Y0 * T:(G_R + 1) * T])
                            nc.vector.tensor_mul(
                                ar3[:, pj:pj + 2, :], g(G_A),
                                yr3[:, pj:pj + 2, 2 * T:3 * T])
                            nc.vector.tensor_scalar_max(
                                rho3[:, pj:pj + 2, :],
                                pm3[:, :, G_Q0 * T:(G_Q1 + 1) * T], 0.0)
                            nc.vector.tensor_mul(
                                tab3[:, pj:pj + 2, 0:T],
                                rho3[:, pj:pj + 2, 0:T], g(G_E0))
                            nc.vector.tensor_mul(
                                tab3[:, pj:pj + 2, T:2 * T],
                                rho3[:, pj:pj + 2, T:2 * T], g(G_E1))

                # ---------------- phase D: supergroup elementwise chain [128, 544]
                rho0 = rho3[:, :, 0:T]
                rho1 = rho3[:, :, T:2 * T]
                y0v = yr3[:, :, 0:T]
                y1v = yr3[:, :, T:2 * T]

                n2 = mtp.tile([P, FW], F32, tag="n2")
                nc.vector.tensor_add(n2[:], ar_t[:], s_sb[:])
                s2 = mtp.tile([P, FW], F32, tag="s2")
                nc.scalar.activation(s2[:], n2[:], AF.Sqrt, bias=prm(P_EPS),
                                     scale=1.0 / D)
                inv2 = mtp.tile([P, FW], F32, tag="inv2")
                nc.vector.reciprocal(inv2[:], s2[:])

                z0 = mtp.tile([P, FW], F32, tag="z0")
                nc.gpsimd.tensor_mul(z0[:], rho0, inv2[:])
                z1 = mtp.tile([P, FW], F32, tag="z1")
                nc.gpsimd.tensor_mul(z1[:], rho1, inv2[:])
                tau = mtp.tile([P, FW], F32, tag="tau")
                nc.gpsimd.tensor_add(tau[:], tab3[:, :, 0:T], tab3[:, :, T:2 * T])
                it2 = mtp.tile([P, FW], F32, tag="it2")
                nc.gpsimd.tensor_mul(it2[:], tau[:], inv2[:])

                v1 = mtp.tile([P, FW], F32, tag="v1")
                nc.vector.scalar_tensor_tensor(v1[:], z1[:], prm(P_RAT), z0[:],
                                               op0=ALU.mult, op1=ALU.add)
                v1sq = mtp.tile([P, FW], F32, tag="v1sq")
                nc.scalar.activation(v1sq[:], v1[:], AF.Square,
                                     bias=prm(P_ZERO), scale=prm(P_SQ0))
                v2sq = mtp.tile([P, FW], F32, tag="v2sq")
                nc.scalar.activation(v2sq[:], z1[:], AF.Square,
                                     bias=prm(P_ZERO), scale=prm(P_C3))

                n3 = mtp.tile([P, FW], F32, tag="n3")
                nc.vector.tensor_add(n3[:], n2[:], it2[:])
                nc.gpsimd.tensor_add(n3[:], n3[:], v1sq[:])
                nc.vector.tensor_add(n3[:], n3[:], v2sq[:])
                s3 = mtp.tile([P, FW], F32, tag="s3")
                nc.scalar.activation(s3[:], n3[:], AF.Sqrt, bias=prm(P_EPS),
                                     scale=1.0 / D)
                inv3 = mtp.tile([P, FW], F32, tag="inv3")
                nc.vector.reciprocal(inv3[:], s3[:])

                lint = mtp.tile([P, SGB * 68], F32, tag="lint")
                lint3 = lint[:].rearrange("p (j c) -> p j c", c=68)
                p0 = mtp.tile([P, FW], F32, tag="p0")
                nc.vector.scalar_tensor_tensor(p0[:], z1[:], prm(P_H10), y0v,
                                               op0=ALU.mult, op1=ALU.add)
                nc.vector.scalar_tensor_tensor(p0[:], z0[:], prm(P_H00), p0[:],
                                               op0=ALU.mult, op1=ALU.add)
                nc.gpsimd.tensor_mul(lint3[:, :, 0:T], p0[:], inv3[:])
                p1 = mtp.tile([P, FW], F32, tag="p1")
                nc.vector.scalar_tensor_tensor(p1[:], z1[:], prm(P_H11), y1v,
                                               op0=ALU.mult, op1=ALU.add)
                nc.vector.scalar_tensor_tensor(p1[:], z0[:], prm(P_H01), p1[:],
                                               op0=ALU.mult, op1=ALU.add)
                nc.gpsimd.tensor_mul(lint3[:, :, T:2 * T], p1[:], inv3[:])

                # ---------------- phase F: final expansion + store
                pt2 = None
                for j in range(SGB):
                    c4 = j % 4
                    if c4 == 0:
                        pt2 = ptp.tile([2 * T, 4 * P], F32, tag="tp")
                    nc.tensor.transpose(pt2[:, c4 * P:(c4 + 1) * P],
                                        lint3[:, j, :], ident[:])
                    if c4 == 3:
                        lt2_t = lt2p.tile([2 * T, 4 * P], F32, tag="lt2")
                        nc.vector.tensor_copy(lt2_t[:], pt2[:])
                        for b in range(4):
                            jb = j - 3 + b
                            po = poutp.tile([P, N2], F32, tag="po")
                            nc.tensor.matmul(po[:], lt2_t[:, b * P:(b + 1) * P],
                                             rhs2_sb[:], start=True, stop=True)
                            o_sb = outp.tile([P, N2], F32, tag="osb")
                            if jb % 2 == 0:
                                nc.vector.tensor_copy(o_sb[:], po[:])
                            else:
                                nc.scalar.copy(o_sb[:], po[:])
                            nc.sync.dma_start(out_v[:, j0 + jb, :], o_sb[:])

    nc.compile()
    return nc


_CACHE = {}


def _get_nc():
    if "nc" not in _CACHE:
        _CACHE["nc"] = build_bass()
    return _CACHE["nc"]


def kernel(**inputs) -> np.ndarray:
    idx = np.asarray(inputs["idx"]).astype(np.int32)
    kw = {k: np.asarray(v, np.float64) for k, v in inputs.items() if k != "idx"}
    R, RHS2, SROW, PRM, CODE = host_tables(**kw)
    idxc = np.ascontiguousarray(CODE[idx])  # remap token id -> angle code
    nc = _get_nc()
    in_maps = [
        {"idx": idxc[c * BC:(c + 1) * BC], "R": R, "RHS2": RHS2,
         "SROW": SROW, "PRM": PRM}
        for c in range(NCORES)
    ]
    res = run_bass_kernel_spmd(nc, in_maps, core_ids=list(range(NCORES)))
    out = np.concatenate([res.results[c]["out"] for c in range(NCORES)], axis=0)
    return np.ascontiguousarray(out.reshape(B, T, V).astype(np.float32))



# revision 8
# speedup vs baseline: 1.8012x; 1.8012x over previous
"""Trainium2 Bass kernel for nn_MicroAdder (tiny dense transformer).

Decomposition (v3):
  Every per-element quantity in the reference network is affine in the basis
  [u_s, w_s, 1] with u = cos(tok_angle), w = sin(tok_angle) and
  position-dependent constant coefficients.  The HOST gathers u/w from a
  14-entry table and ships the basis directly as fp16 [69, B] -- no on-device
  trig, no int->float casts, and the basis is already transposed so mm1 needs
  no PE transpose.  One PE matmul per 128-row block (lhsT = basis slice,
  rhs = R [69, 272]) produces all 8 linear forms (att, rr, q0, q1, e0, e1,
  y0, y1), with the causal-softmax attention mixing folded into R.  A short
  fp16 elementwise chain (DVE 2x modes; Ln/Exp for the two rsqrt's -- the DVE
  reciprocal is ~8 cycles/elem, Ln+Exp on ACT is ~3x cheaper) produces the two
  logit forms L0, L1.  These are DMA-transposed (XBAR) and expanded to the
  (T,V) logits by a second PE matmul with a block-diagonal constant rhs, then
  stored as bf16 (host converts to fp32).

Sharding: pure data parallel over the batch dim across 8 NeuronCores.
"""

import math
import sys

import numpy as np

for _p in ("/opt/trn_rl_repo", "/root/.axon_site/_ro/trn_rl_repo"):
    if _p not in sys.path:
        sys.path.append(_p)

import concourse.bacc as bacc  # noqa: E402
import concourse.bass as bass  # noqa: E402
import concourse.tile as tile  # noqa: E402
from concourse import mybir  # noqa: E402
from concourse.bass_utils import run_bass_kernel_spmd  # noqa: E402

# ---------------------------------------------------------------- problem dims
B, T, V = 65536, 34, 14
D, EPS, MAX_DIGITS = 5, 1e-5, 10
NCORES = 8
BC = B // NCORES            # rows per core = 8192
P = 128                     # partitions
NBLK = BC // P              # 64 blocks of 128 rows per core
SGB = 16                    # blocks per supergroup
NSG = NBLK // SGB           # 4 supergroups
NQ = SGB // 4               # quads per supergroup = 4
K1 = 2 * T + 1              # basis size = 69
NG = 8                      # mm1 form groups
N1 = NG * T                 # 272
N2 = T * V                  # 476
FW = SGB * T                # 544 chain columns per supergroup
NPRM = 9

F32 = mybir.dt.float32
F16 = mybir.dt.float16
BF16 = mybir.dt.bfloat16
I32 = mybir.dt.int32
AF = mybir.ActivationFunctionType
ALU = mybir.AluOpType

# mm1 output group order (columns g*T..(g+1)*T of R).  ATT/Y0/Y1 adjacent so
# one ACT copy drains all three; the att*rr product then has only one PSUM
# operand (HW allows at most one non-scalar PSUM input per vector op).
G_ATT, G_Y0, G_Y1, G_RR, G_Q0, G_Q1, G_E0, G_E1 = range(8)

# PRM slots
P_RAT, P_SQ0, P_C3, P_H00, P_H10, P_H01, P_H11, P_EPS, P_ZERO = range(9)


# ---------------------------------------------------------------- host tables
def host_tables(tok_A, tok_start, tok_stride, sp_amp, sp_phase, sp_slope, sp_offset,
                norm_w, q_w, q_phase, out_A, out_B, fc1_w, fc2_w, head_w):
    f = np.float64
    A = f(tok_A)
    t = np.arange(T, dtype=f)
    th = 2.0 * np.pi * t / MAX_DIGITS + f(sp_phase)
    pos = np.stack([f(sp_amp) * np.cos(th), f(sp_amp) * np.sin(th),
                    f(sp_slope) * t + f(sp_offset)], axis=-1)
    k = pos @ np.asarray(q_w, f).T
    c0, s0 = np.cos(f(q_phase[0])), np.sin(f(q_phase[0]))
    q = k.copy()
    q[:, 0] = c0 * k[:, 0] - s0 * k[:, 1]
    q[:, 1] = s0 * k[:, 0] + c0 * k[:, 1]
    scores = (q @ k.T) / np.sqrt(f(5.0))
    sm = np.where(np.tril(np.ones((T, T), bool)), scores, -np.inf)
    sm = sm - sm.max(-1, keepdims=True)
    e = np.exp(sm)
    attn = e / e.sum(-1, keepdims=True)

    nw = np.asarray(norm_w, f)
    oA = np.asarray(out_A, f)[:, 0]
    oB = np.asarray(out_B, f)[0]
    S_t = A * A + (pos ** 2).sum(-1)          # |x|^2 per position (tok part = A^2)
    rms1 = np.sqrt(S_t / D + EPS)

    M0 = attn * (A * nw[0] * oA[0] / rms1)[None, :]
    M1 = attn * (A * nw[1] * oA[1] / rms1)[None, :]
    c_t = attn @ ((pos * (nw[2:] * oA[2:])[None, :]).sum(-1) / rms1)

    g0 = np.asarray(fc2_w, f)[:, 0]
    g1 = np.asarray(fc2_w, f)[:, 1]
    projs = {
        G_Q0: nw * np.asarray(fc1_w, f)[0],
        G_Q1: nw * np.asarray(fc1_w, f)[1],
        G_E0: 2.0 * g0,
        G_E1: 2.0 * g1,
        G_Y0: nw * np.asarray(head_w, f)[0],
        G_Y1: nw * np.asarray(head_w, f)[1],
    }
    # R columns: group g covers cols g*T..(g+1)*T; rows: u_s (0:T), w_s (T:2T),
    # const (2T).  att-form = z (attention scalar), rr-form = 2 x.oB + |oB|^2 z.
    R = np.zeros((K1, NG * T), dtype=f)
    dd = np.eye(T, dtype=f)
    b2 = (oB ** 2).sum()
    for gi in range(NG):
        cols = slice(gi * T, (gi + 1) * T)
        if gi == G_ATT:
            R[0:T, cols] = M0.T
            R[T:2 * T, cols] = M1.T
            R[2 * T, cols] = c_t
        elif gi == G_RR:
            R[0:T, cols] = 2 * A * oB[0] * dd + b2 * M0.T
            R[T:2 * T, cols] = 2 * A * oB[1] * dd + b2 * M1.T
            R[2 * T, cols] = 2 * (pos * oB[None, 2:]).sum(-1) + b2 * c_t
        else:
            v = projs[gi]
            bv = (oB * v).sum()
            R[0:T, cols] = A * v[0] * dd + bv * M0.T
            R[T:2 * T, cols] = A * v[1] * dd + bv * M1.T
            R[2 * T, cols] = (pos * v[None, 2:]).sum(-1) + bv * c_t

    G00, G01, G11 = (g0 * g0).sum(), (g0 * g1).sum(), (g1 * g1).sum()
    if G00 > 1e-30:
        sq0, rat = np.sqrt(G00), G01 / G00
        c3 = np.sqrt(max(G11 - G01 * G01 / G00, 0.0))
    else:
        sq0, rat, c3 = 0.0, 0.0, np.sqrt(G11)
    hv0 = nw * np.asarray(head_w, f)[0]
    hv1 = nw * np.asarray(head_w, f)[1]
    H = np.array([[(g0 * hv0).sum(), (g0 * hv1).sum()],
                  [(g1 * hv0).sum(), (g1 * hv1).sum()]])

    dvoc = np.arange(V, dtype=f)
    ang = f(tok_start) + dvoc * f(tok_stride)
    E = np.stack([A * np.cos(ang), A * np.sin(ang)], axis=-1)   # (V, 2)
    RHS2 = np.zeros((2 * T, N2), dtype=f)
    for t_ in range(T):
        RHS2[t_, t_ * V:(t_ + 1) * V] = E[:, 0]
        RHS2[T + t_, t_ * V:(t_ + 1) * V] = E[:, 1]

    # token basis tables: u = cos(ang_v), w = sin(ang_v)
    CU = np.cos(ang)
    SW = np.sin(ang)

    SROW = np.tile(S_t, SGB)[None, :]  # [1, 544]
    PRM = np.zeros((1, NPRM), dtype=f)
    PRM[0, P_RAT] = rat
    PRM[0, P_SQ0] = sq0
    PRM[0, P_C3] = c3
    PRM[0, P_H00] = H[0, 0]
    PRM[0, P_H10] = H[1, 0]
    PRM[0, P_H01] = H[0, 1]
    PRM[0, P_H11] = H[1, 1]
    PRM[0, P_EPS] = EPS
    PRM[0, P_ZERO] = 0.0
    return (R.astype(np.float16), RHS2.astype(np.float16),
            np.ascontiguousarray(SROW, np.float16).copy(),
            np.ascontiguousarray(PRM, np.float32).copy(),
            CU.astype(np.float16), SW.astype(np.float16))


def build_basis(idx, CU, SW):
    """[69, B] fp16: rows 0:T = u_t(b), rows T:2T = w_t(b), row 2T = 1."""
    bset = np.empty((K1, idx.shape[0]), np.float16)
    bset[0:T, :] = CU[idx].T
    bset[T:2 * T, :] = SW[idx].T
    bset[2 * T, :] = np.float16(1.0)
    return bset


# ---------------------------------------------------------------- bass kernel
def build_bass():
    nc = bacc.Bacc("TRN2", target_bir_lowering=False, debug=False)

    basis_d = nc.dram_tensor("basis", [K1, BC], F16, kind="ExternalInput").ap()
    r_d = nc.dram_tensor("R", [K1, N1], F16, kind="ExternalInput").ap()
    rhs2_d = nc.dram_tensor("RHS2", [2 * T, N2], F16, kind="ExternalInput").ap()
    srow_d = nc.dram_tensor("SROW", [1, FW], F16, kind="ExternalInput").ap()
    prm_d = nc.dram_tensor("PRM", [1, NPRM], F32, kind="ExternalInput").ap()
    out_d = nc.dram_tensor("out", [BC, N2], BF16, kind="ExternalOutput").ap()

    # DRAM out view: block n covers batch rows n*128..n*128+127, partition p
    # holds row n*128+p (matches mm1 lhsT = basis columns n*128+p)
    out_v = out_d.rearrange("(n p) c -> p n c", p=P)       # [128, 64, 476]

    with tile.TileContext(nc) as tc:
        with (
            tc.tile_pool(name="const", bufs=1) as cpool,
            tc.tile_pool(name="bas", bufs=2) as basp,
            tc.tile_pool(name="dr", bufs=2) as drp,
            tc.tile_pool(name="ch", bufs=2) as chp,
            tc.tile_pool(name="ltT", bufs=4) as ltp,
            tc.tile_pool(name="outsb", bufs=3) as outp,
            tc.tile_pool(name="pmm1", bufs=1, space="PSUM") as pmm1p,
            tc.tile_pool(name="pout", bufs=1, space="PSUM") as poutp,
        ):
            # ---- constants
            r_sb = cpool.tile([K1, N1], F16)
            nc.sync.dma_start(r_sb[:], r_d)
            rhs2_sb = cpool.tile([2 * T, N2], F16)
            nc.sync.dma_start(rhs2_sb[:], rhs2_d)
            s_sb = cpool.tile([P, FW], F16)
            nc.sync.dma_start(s_sb[:], srow_d.broadcast_to([P, FW]))
            prm_sb = cpool.tile([P, NPRM], F32)
            nc.sync.dma_start(prm_sb[:], prm_d.broadcast_to([P, NPRM]))

            def prm(i):
                return prm_sb[:, i:i + 1]

            # persistent double-buffered L-form tiles; cols 68:128 of each
            # block stay zero (transposed padding rows are never read by mm2,
            # but the XBAR transpose reads them)
            lints = [cpool.tile([P, SGB * P], F16, tag=f"lint{i}", name=f"lint{i}")
                     for i in range(2)]
            for lt in lints:
                nc.vector.memset(lt[:], 0.0)

            # state carried between supergroups for the pipelined F phase
            prev = None  # (lint3 view, j0 of previous supergroup)

            def emit_F_quad(lint3, j0, q):
                """Transpose + mm2 + drain + store for blocks q*4..q*4+3."""
                po = poutp.tile([P, 4 * 512], F32, tag="po")
                po4 = po[:].rearrange("p (k c) -> p k c", k=4)
                ltT = []
                for k in range(4):
                    j = q * 4 + k
                    lt = ltp.tile([P, P], F16, tag="ltT")
                    nc.sync.dma_start_transpose(lt[:], lint3[:, j, :])
                    ltT.append(lt)
                for k in range(4):
                    nc.tensor.matmul(po4[:, k, 0:N2], ltT[k][0:2 * T, :],
                                     rhs2_sb[:], start=True, stop=True)
                o_sb = outp.tile([P, 4, N2], BF16, tag="osb")
                eng = (nc.vector, nc.scalar, nc.vector, nc.scalar)[q]
                if eng is nc.scalar:
                    nc.scalar.copy(o_sb[:], po4[:, :, 0:N2])
                else:
                    eng.tensor_copy(o_sb[:], po4[:, :, 0:N2])
                nc.sync.dma_start(out_v[:, j0 + q * 4:j0 + q * 4 + 4, :], o_sb[:])

            for sg in range(NSG):
                j0 = sg * SGB
                lint = lints[sg % 2]
                lint3 = lint[:].rearrange("p (j c) -> p j c", c=P)

                # ---------------- phase A: basis DMA  [69, 2048]
                b_t = basp.tile([K1, SGB * P], F16, tag="bas")
                nc.sync.dma_start(b_t[:], basis_d[:, j0 * P:(j0 + SGB) * P])

                # drain targets for the whole supergroup (fp16)
                ayy = drp.tile([P, SGB, 3 * T], F16, tag="ayy")
                ar = drp.tile([P, SGB, T], F16, tag="ar")
                rho = drp.tile([P, SGB, 2 * T], F16, tag="rho")
                tab = drp.tile([P, SGB, 2 * T], F16, tag="tab")

                # ---------------- phase B: mm1 per quad + drains
                for q in range(NQ):
                    pm = pmm1p.tile([P, 4 * 512], F32, tag="mm1")
                    pm4 = pm[:].rearrange("p (k c) -> p k c", k=4)
                    for k in range(4):
                        j = q * 4 + k
                        nc.tensor.matmul(
                            pm4[:, k, 0:N1],
                            b_t[:, j * P:(j + 1) * P],
                            r_sb[:], start=True, stop=True)
                    qs = slice(q * 4, q * 4 + 4)

                    def g(g0, g1=None):
                        g1 = g0 if g1 is None else g1
                        return pm4[:, :, g0 * T:(g1 + 1) * T]

                    nc.scalar.copy(ayy[:, qs, :], g(G_ATT, G_Y1))
                    nc.vector.tensor_mul(ar[:, qs, :], ayy[:, qs, 0:T], g(G_RR))
                    nc.vector.tensor_scalar_max(rho[:, qs, :], g(G_Q0, G_Q1), 0.0)
                    nc.vector.tensor_mul(tab[:, qs, :], rho[:, qs, :], g(G_E0, G_E1))

                    # pipelined phase F of the previous supergroup
                    if prev is not None:
                        emit_F_quad(prev[0], prev[1], q)

                # ---------------- phase D: fp16 chain on [128, 544]
                arf = ar[:].rearrange("p j t -> p (j t)")
                rho0 = rho[:, :, 0:T]
                rho1 = rho[:, :, T:2 * T]
                y0 = ayy[:, :, T:2 * T]
                y1 = ayy[:, :, 2 * T:3 * T]

                def ct(tag):
                    t_ = chp.tile([P, FW], F16, tag=tag)
                    return t_, t_[:].rearrange("p (j t) -> p j t", t=T)

                n2, n2v = ct("n2")
                nc.vector.tensor_add(n2[:], arf, s_sb[:])
                ln2, _ = ct("ln2")
                nc.scalar.activation(ln2[:], n2[:], AF.Ln, bias=prm(P_EPS), scale=1.0 / D)
                inv2, inv2v = ct("inv2")
                nc.scalar.activation(inv2[:], ln2[:], AF.Exp, bias=prm(P_ZERO), scale=-0.5)

                z0, z0v = ct("z0")
                nc.gpsimd.tensor_mul(z0v, rho0, inv2v)
                z1, z1v = ct("z1")
                nc.gpsimd.tensor_mul(z1v, rho1, inv2v)
                tau, tauv = ct("tau")
                nc.gpsimd.tensor_add(tauv, tab[:, :, 0:T], tab[:, :, T:2 * T])
                it2, it2v = ct("it2")
                nc.gpsimd.tensor_mul(it2[:], tau[:], inv2[:])

                v1, _ = ct("v1")
                nc.vector.tensor_scalar(v1[:], z1[:], prm(P_RAT), None, op0=ALU.mult)
                nc.vector.tensor_add(v1[:], v1[:], z0[:])
                q1s, _ = ct("q1s")
                nc.scalar.activation(q1s[:], v1[:], AF.Square, bias=prm(P_ZERO),
                                     scale=prm(P_SQ0))
                q2s, _ = ct("q2s")
                nc.scalar.activation(q2s[:], z1[:], AF.Square, bias=prm(P_ZERO),
                                     scale=prm(P_C3))

                n3, _ = ct("n3")
                nc.vector.tensor_add(n3[:], n2[:], it2[:])
                nc.vector.tensor_add(n3[:], n3[:], q1s[:])
                nc.vector.tensor_add(n3[:], n3[:], q2s[:])
                ln3, _ = ct("ln3")
                nc.scalar.activation(ln3[:], n3[:], AF.Ln, bias=prm(P_EPS), scale=1.0 / D)
                inv3, inv3v = ct("inv3")
                nc.scalar.activation(inv3[:], ln3[:], AF.Exp, bias=prm(P_ZERO), scale=-0.5)

                zh0, _ = ct("zh0")
                zh1, _ = ct("zh1")
                p0, p0v = ct("p0")
                p1, p1v = ct("p1")
                nc.vector.tensor_scalar(zh0[:], z0[:], prm(P_H00), None, op0=ALU.mult)
                nc.vector.tensor_scalar(zh1[:], z1[:], prm(P_H10), None, op0=ALU.mult)
                nc.vector.tensor_add(p0v, y0, zh0[:].rearrange("p (j t) -> p j t", t=T))
                nc.vector.tensor_add(p0[:], p0[:], zh1[:])
                nc.vector.tensor_scalar(zh0[:], z0[:], prm(P_H01), None, op0=ALU.mult)
                nc.vector.tensor_scalar(zh1[:], z1[:], prm(P_H11), None, op0=ALU.mult)
                nc.vector.tensor_add(p1v, y1, zh0[:].rearrange("p (j t) -> p j t", t=T))
                nc.vector.tensor_add(p1[:], p1[:], zh1[:])

                nc.vector.tensor_mul(lint3[:, :, 0:T], p0v, inv3v)
                nc.vector.tensor_mul(lint3[:, :, T:2 * T], p1v, inv3v)

                prev = (lint3, j0)

            # drain the last supergroup's F phase
            for q in range(NQ):
                emit_F_quad(prev[0], prev[1], q)

    nc.compile()
    return nc


_CACHE = {}


def _get_nc():
    if "nc" not in _CACHE:
        _CACHE["nc"] = build_bass()
    return _CACHE["nc"]


def kernel(**inputs) -> np.ndarray:
    idx = np.asarray(inputs["idx"]).astype(np.int64)
    kw = {k: np.asarray(v, np.float64) for k, v in inputs.items() if k != "idx"}
    R, RHS2, SROW, PRM, CU, SW = host_tables(**kw)
    nc = _get_nc()
    in_maps = []
    for c in range(NCORES):
        bas = build_basis(idx[c * BC:(c + 1) * BC], CU, SW)
        in_maps.append({"basis": bas, "R": R, "RHS2": RHS2,
                        "SROW": SROW, "PRM": PRM})
    res = run_bass_kernel_spmd(nc, in_maps, core_ids=list(range(NCORES)))
    out = np.concatenate([np.asarray(res.results[c]["out"]).astype(np.float32)
                          for c in range(NCORES)], axis=0)
    return np.ascontiguousarray(out.reshape(B, T, V))


# revision 9
# speedup vs baseline: 2.5745x; 1.4293x over previous
"""Trainium2 Bass kernel for nn_MicroAdder (tiny dense transformer).

Decomposition (v3):
  Every per-element quantity in the reference network is affine in the basis
  [u_s, w_s, 1] with u = cos(tok_angle), w = sin(tok_angle) and
  position-dependent constant coefficients.  The HOST gathers u/w from a
  14-entry table and ships the basis directly as fp16 [69, B] -- no on-device
  trig, no int->float casts, and the basis is already transposed so mm1 needs
  no PE transpose.  One PE matmul per 128-row block (lhsT = basis slice,
  rhs = R [69, 272]) produces all 8 linear forms (att, rr, q0, q1, e0, e1,
  y0, y1), with the causal-softmax attention mixing folded into R.  A short
  fp16 elementwise chain (DVE 2x modes; Ln/Exp for the two rsqrt's -- the DVE
  reciprocal is ~8 cycles/elem, Ln+Exp on ACT is ~3x cheaper) produces the two
  logit forms L0, L1.  These are DMA-transposed (XBAR) and expanded to the
  (T,V) logits by a second PE matmul with a block-diagonal constant rhs, then
  stored as bf16 (host converts to fp32).

Sharding: pure data parallel over the batch dim across 8 NeuronCores.
"""

import math
import sys

import numpy as np

for _p in ("/opt/trn_rl_repo", "/root/.axon_site/_ro/trn_rl_repo"):
    if _p not in sys.path:
        sys.path.append(_p)

import concourse.bacc as bacc  # noqa: E402
import concourse.bass as bass  # noqa: E402
import concourse.tile as tile  # noqa: E402
from concourse import mybir  # noqa: E402
from concourse.bass_utils import run_bass_kernel_spmd  # noqa: E402
from concourse.masks import make_identity  # noqa: E402

# ---------------------------------------------------------------- problem dims
B, T, V = 65536, 34, 14
D, EPS, MAX_DIGITS = 5, 1e-5, 10
NCORES = 8
BC = B // NCORES            # rows per core = 8192
P = 128                     # partitions
NBLK = BC // P              # 64 blocks of 128 rows per core
SGB = 16                    # blocks per supergroup
NSG = NBLK // SGB           # 4 supergroups
NQ = SGB // 4               # quads per supergroup = 4
K1 = 2 * T + 1              # basis size = 69
NG = 8                      # mm1 form groups
N1 = NG * T                 # 272
N2 = T * V                  # 476
FW = SGB * T                # 544 chain columns per supergroup
NPRM = 9

F32 = mybir.dt.float32
F16 = mybir.dt.float16
BF16 = mybir.dt.bfloat16
I32 = mybir.dt.int32
AF = mybir.ActivationFunctionType
ALU = mybir.AluOpType

# mm1 output group order (columns g*T..(g+1)*T of R).  ATT/Y0/Y1 adjacent so
# one ACT copy drains all three; the att*rr product then has only one PSUM
# operand (HW allows at most one non-scalar PSUM input per vector op).
G_ATT, G_Y0, G_Y1, G_RR, G_Q0, G_Q1, G_E0, G_E1 = range(8)

# PRM slots
P_RAT, P_SQ0, P_C3, P_H00, P_H10, P_H01, P_H11, P_EPS, P_ZERO = range(9)


# ---------------------------------------------------------------- host tables
def host_tables(tok_A, tok_start, tok_stride, sp_amp, sp_phase, sp_slope, sp_offset,
                norm_w, q_w, q_phase, out_A, out_B, fc1_w, fc2_w, head_w):
    f = np.float64
    A = f(tok_A)
    t = np.arange(T, dtype=f)
    th = 2.0 * np.pi * t / MAX_DIGITS + f(sp_phase)
    pos = np.stack([f(sp_amp) * np.cos(th), f(sp_amp) * np.sin(th),
                    f(sp_slope) * t + f(sp_offset)], axis=-1)
    k = pos @ np.asarray(q_w, f).T
    c0, s0 = np.cos(f(q_phase[0])), np.sin(f(q_phase[0]))
    q = k.copy()
    q[:, 0] = c0 * k[:, 0] - s0 * k[:, 1]
    q[:, 1] = s0 * k[:, 0] + c0 * k[:, 1]
    scores = (q @ k.T) / np.sqrt(f(5.0))
    sm = np.where(np.tril(np.ones((T, T), bool)), scores, -np.inf)
    sm = sm - sm.max(-1, keepdims=True)
    e = np.exp(sm)
    attn = e / e.sum(-1, keepdims=True)

    nw = np.asarray(norm_w, f)
    oA = np.asarray(out_A, f)[:, 0]
    oB = np.asarray(out_B, f)[0]
    S_t = A * A + (pos ** 2).sum(-1)          # |x|^2 per position (tok part = A^2)
    rms1 = np.sqrt(S_t / D + EPS)

    M0 = attn * (A * nw[0] * oA[0] / rms1)[None, :]
    M1 = attn * (A * nw[1] * oA[1] / rms1)[None, :]
    c_t = attn @ ((pos * (nw[2:] * oA[2:])[None, :]).sum(-1) / rms1)

    g0 = np.asarray(fc2_w, f)[:, 0]
    g1 = np.asarray(fc2_w, f)[:, 1]
    projs = {
        G_Q0: nw * np.asarray(fc1_w, f)[0],
        G_Q1: nw * np.asarray(fc1_w, f)[1],
        G_E0: 2.0 * g0,
        G_E1: 2.0 * g1,
        G_Y0: nw * np.asarray(head_w, f)[0],
        G_Y1: nw * np.asarray(head_w, f)[1],
    }
    # R columns: group g covers cols g*T..(g+1)*T; rows: u_s (0:T), w_s (T:2T),
    # const (2T).  att-form = z (attention scalar), rr-form = 2 x.oB + |oB|^2 z.
    R = np.zeros((K1, NG * T), dtype=f)
    dd = np.eye(T, dtype=f)
    b2 = (oB ** 2).sum()
    for gi in range(NG):
        cols = slice(gi * T, (gi + 1) * T)
        if gi == G_ATT:
            R[0:T, cols] = M0.T
            R[T:2 * T, cols] = M1.T
            R[2 * T, cols] = c_t
        elif gi == G_RR:
            R[0:T, cols] = 2 * A * oB[0] * dd + b2 * M0.T
            R[T:2 * T, cols] = 2 * A * oB[1] * dd + b2 * M1.T
            R[2 * T, cols] = 2 * (pos * oB[None, 2:]).sum(-1) + b2 * c_t
        else:
            v = projs[gi]
            bv = (oB * v).sum()
            R[0:T, cols] = A * v[0] * dd + bv * M0.T
            R[T:2 * T, cols] = A * v[1] * dd + bv * M1.T
            R[2 * T, cols] = (pos * v[None, 2:]).sum(-1) + bv * c_t

    G00, G01, G11 = (g0 * g0).sum(), (g0 * g1).sum(), (g1 * g1).sum()
    if G00 > 1e-30:
        sq0, rat = np.sqrt(G00), G01 / G00
        c3 = np.sqrt(max(G11 - G01 * G01 / G00, 0.0))
    else:
        sq0, rat, c3 = 0.0, 0.0, np.sqrt(G11)
    hv0 = nw * np.asarray(head_w, f)[0]
    hv1 = nw * np.asarray(head_w, f)[1]
    H = np.array([[(g0 * hv0).sum(), (g0 * hv1).sum()],
                  [(g1 * hv0).sum(), (g1 * hv1).sum()]])

    dvoc = np.arange(V, dtype=f)
    ang = f(tok_start) + dvoc * f(tok_stride)
    E = np.stack([A * np.cos(ang), A * np.sin(ang)], axis=-1)   # (V, 2)
    RHS2 = np.zeros((2 * T, N2), dtype=f)
    for t_ in range(T):
        RHS2[t_, t_ * V:(t_ + 1) * V] = E[:, 0]
        RHS2[T + t_, t_ * V:(t_ + 1) * V] = E[:, 1]

    # token basis tables: u = cos(ang_v), w = sin(ang_v)
    CU = np.cos(ang)
    SW = np.sin(ang)

    SROW = np.tile(S_t, SGB)[None, :]  # [1, 544]
    PRM = np.zeros((1, NPRM), dtype=f)
    PRM[0, P_RAT] = rat
    PRM[0, P_SQ0] = sq0
    PRM[0, P_C3] = c3
    PRM[0, P_H00] = H[0, 0]
    PRM[0, P_H10] = H[1, 0]
    PRM[0, P_H01] = H[0, 1]
    PRM[0, P_H11] = H[1, 1]
    PRM[0, P_EPS] = EPS
    PRM[0, P_ZERO] = 0.0
    return (R.astype(np.float16), RHS2.astype(np.float16),
            np.ascontiguousarray(SROW, np.float16).copy(),
            np.ascontiguousarray(PRM, np.float32).copy(),
            CU.astype(np.float16), SW.astype(np.float16))


def build_basis(idx, CU, SW):
    """[69, B] fp16: rows 0:T = u_t(b), rows T:2T = w_t(b), row 2T = 1."""
    bset = np.empty((K1, idx.shape[0]), np.float16)
    bset[0:T, :] = CU[idx].T
    bset[T:2 * T, :] = SW[idx].T
    bset[2 * T, :] = np.float16(1.0)
    return bset


def act_raw(nc, out, in_, func, bias, scale):
    """Emit InstActivation directly (same lowering as BassScalarEngine.
    activation) for funcs the wrapper refuses (Rsqrt)."""
    se = nc.scalar
    inputs = [se.lower_ap(in_)]
    for arg in (bias, scale, 0.0):
        if isinstance(arg, bass.AP):
            inputs.append(se.lower_ap(arg))
        else:
            inputs.append(mybir.ImmediateValue(dtype=mybir.dt.float32, value=arg))
    return se.add_instruction(mybir.InstActivation(
        name=se.bass.get_next_instruction_name(),
        func=func, ins=inputs, outs=[se.lower_ap(out)]))


# ---------------------------------------------------------------- bass kernel
def build_bass():
    nc = bacc.Bacc("TRN2", target_bir_lowering=False, debug=False)

    basis_d = nc.dram_tensor("basis", [K1, BC], F16, kind="ExternalInput").ap()
    r_d = nc.dram_tensor("R", [K1, N1], F16, kind="ExternalInput").ap()
    rhs2_d = nc.dram_tensor("RHS2", [2 * T, N2], F16, kind="ExternalInput").ap()
    srow_d = nc.dram_tensor("SROW", [1, FW], F16, kind="ExternalInput").ap()
    prm_d = nc.dram_tensor("PRM", [1, NPRM], F32, kind="ExternalInput").ap()
    out_d = nc.dram_tensor("out", [BC, N2], BF16, kind="ExternalOutput").ap()

    # DRAM out view: block n covers batch rows n*128..n*128+127, partition p
    # holds row n*128+p (matches mm1 lhsT = basis columns n*128+p)
    out_v = out_d.rearrange("(n p) c -> p n c", p=P)       # [128, 64, 476]

    with tile.TileContext(nc) as tc:
        with (
            tc.tile_pool(name="const", bufs=1) as cpool,
            tc.tile_pool(name="bas", bufs=2) as basp,
            tc.tile_pool(name="dr", bufs=2) as drp,
            tc.tile_pool(name="ch", bufs=2) as chp,
            tc.tile_pool(name="ltT", bufs=4) as ltp,
            tc.tile_pool(name="outsb", bufs=3) as outp,
            tc.tile_pool(name="pmm1", bufs=1, space="PSUM") as pmm1p,
            tc.tile_pool(name="pout", bufs=1, space="PSUM") as poutp,
            tc.tile_pool(name="ptr", bufs=2, space="PSUM") as ptrp,
        ):
            # ---- constants
            ident = cpool.tile([P, P], F16)
            make_identity(nc, ident[:])
            r_sb = cpool.tile([K1, N1], F16)
            nc.sync.dma_start(r_sb[:], r_d)
            rhs2_sb = cpool.tile([2 * T, N2], F16)
            nc.sync.dma_start(rhs2_sb[:], rhs2_d)
            s_sb = cpool.tile([P, FW], F16)
            nc.sync.dma_start(s_sb[:], srow_d.broadcast_to([P, FW]))
            prm_sb = cpool.tile([P, NPRM], F32)
            nc.sync.dma_start(prm_sb[:], prm_d.broadcast_to([P, NPRM]))

            def prm(i):
                return prm_sb[:, i:i + 1]

            # persistent double-buffered L-form tiles; cols 68:128 of each
            # block stay zero (transposed padding rows are never read by mm2,
            # but the XBAR transpose reads them)
            lints = [cpool.tile([P, SGB * P], F16, tag=f"lint{i}", name=f"lint{i}")
                     for i in range(2)]
            for lt in lints:
                nc.vector.memset(lt[:], 0.0)

            # state carried between supergroups for the pipelined F phase
            prev = None  # (lint3 view, j0 of previous supergroup)

            def emit_F_quad(lint3, j0, q):
                """Transpose + mm2 + drain + store for blocks q*4..q*4+3."""
                pt = ptrp.tile([P, 4 * P], F16, tag="pt")
                pt4 = pt[:].rearrange("p (k c) -> p k c", k=4)
                for k in range(4):
                    nc.tensor.transpose(pt4[:, k, :], lint3[:, q * 4 + k, :],
                                        ident[:])
                lt = ltp.tile([P, 4 * P], F16, tag="ltT")
                nc.vector.tensor_copy(lt[:], pt[:])
                lt4 = lt[:].rearrange("p (k c) -> p k c", k=4)
                for h in range(2):
                    po = poutp.tile([P, 2 * 512], F32, tag="po")
                    po2 = po[:].rearrange("p (k c) -> p k c", k=2)
                    for k in range(2):
                        nc.tensor.matmul(po2[:, k, 0:N2],
                                         lt4[0:2 * T, 2 * h + k, :],
                                         rhs2_sb[:], start=True, stop=True)
                    o_sb = outp.tile([P, 2, N2], BF16, tag="osb")
                    eng = (nc.vector, nc.scalar)[(2 * q + h) % 2]
                    if eng is nc.scalar:
                        nc.scalar.copy(o_sb[:], po2[:, :, 0:N2])
                    else:
                        eng.tensor_copy(o_sb[:], po2[:, :, 0:N2])
                    jb = j0 + q * 4 + 2 * h
                    nc.sync.dma_start(out_v[:, jb:jb + 2, :], o_sb[:])

            for sg in range(NSG):
                j0 = sg * SGB
                lint = lints[sg % 2]
                lint3 = lint[:].rearrange("p (j c) -> p j c", c=P)

                # ---------------- phase A: basis DMA  [69, 2048]
                b_t = basp.tile([K1, SGB * P], F16, tag="bas")
                nc.sync.dma_start(b_t[:], basis_d[:, j0 * P:(j0 + SGB) * P])

                # drain targets for the whole supergroup (fp16)
                ayy = drp.tile([P, SGB, 3 * T], F16, tag="ayy")
                ar = drp.tile([P, SGB, T], F16, tag="ar")
                rho = drp.tile([P, SGB, 2 * T], F16, tag="rho")
                tab = drp.tile([P, SGB, 2 * T], F16, tag="tab")

                # ---------------- phase B: mm1 per quad + drains
                for q in range(NQ):
                    pm = pmm1p.tile([P, 4 * 512], F32, tag="mm1")
                    pm4 = pm[:].rearrange("p (k c) -> p k c", k=4)
                    for k in range(4):
                        j = q * 4 + k
                        nc.tensor.matmul(
                            pm4[:, k, 0:N1],
                            b_t[:, j * P:(j + 1) * P],
                            r_sb[:], start=True, stop=True)
                    qs = slice(q * 4, q * 4 + 4)

                    def g(g0, g1=None):
                        g1 = g0 if g1 is None else g1
                        return pm4[:, :, g0 * T:(g1 + 1) * T]

                    nc.scalar.copy(ayy[:, qs, :], g(G_ATT, G_Y1))
                    nc.vector.tensor_mul(ar[:, qs, :], ayy[:, qs, 0:T], g(G_RR))
                    nc.vector.tensor_scalar_max(rho[:, qs, :], g(G_Q0, G_Q1), 0.0)
                    nc.vector.tensor_mul(tab[:, qs, :], rho[:, qs, :], g(G_E0, G_E1))

                    # pipelined phase F of the previous supergroup
                    if prev is not None:
                        emit_F_quad(prev[0], prev[1], q)

                # ---------------- phase D: fp16 chain on [128, 544]
                arf = ar[:].rearrange("p j t -> p (j t)")
                rho0 = rho[:, :, 0:T]
                rho1 = rho[:, :, T:2 * T]
                y0 = ayy[:, :, T:2 * T]
                y1 = ayy[:, :, 2 * T:3 * T]

                def ct(tag):
                    t_ = chp.tile([P, FW], F16, tag=tag)
                    return t_, t_[:].rearrange("p (j t) -> p j t", t=T)

                n2, n2v = ct("n2")
                nc.gpsimd.tensor_add(n2[:], arf, s_sb[:])
                inv2, inv2v = ct("inv2")
                act_raw(nc, inv2[:], n2[:], AF.Rsqrt, prm(P_EPS), 1.0 / D)

                z0, z0v = ct("z0")
                nc.gpsimd.tensor_mul(z0v, rho0, inv2v)
                z1, z1v = ct("z1")
                nc.gpsimd.tensor_mul(z1v, rho1, inv2v)
                tau, tauv = ct("tau")
                nc.gpsimd.tensor_add(tauv, tab[:, :, 0:T], tab[:, :, T:2 * T])
                it2, it2v = ct("it2")
                nc.gpsimd.tensor_mul(it2[:], tau[:], inv2[:])

                v1, _ = ct("v1")
                nc.vector.tensor_scalar(v1[:], z1[:], prm(P_RAT), None, op0=ALU.mult)
                nc.vector.tensor_add(v1[:], v1[:], z0[:])
                q1s, _ = ct("q1s")
                nc.scalar.activation(q1s[:], v1[:], AF.Square, bias=prm(P_ZERO),
                                     scale=prm(P_SQ0))
                q2s, _ = ct("q2s")
                nc.scalar.activation(q2s[:], z1[:], AF.Square, bias=prm(P_ZERO),
                                     scale=prm(P_C3))

                n3, _ = ct("n3")
                nc.gpsimd.tensor_add(n3[:], n2[:], it2[:])
                nc.gpsimd.tensor_add(n3[:], n3[:], q1s[:])
                nc.gpsimd.tensor_add(n3[:], n3[:], q2s[:])
                inv3, inv3v = ct("inv3")
                act_raw(nc, inv3[:], n3[:], AF.Rsqrt, prm(P_EPS), 1.0 / D)

                zh0, _ = ct("zh0")
                zh1, _ = ct("zh1")
                p0, p0v = ct("p0")
                p1, p1v = ct("p1")
                nc.vector.tensor_scalar(zh0[:], z0[:], prm(P_H00), None, op0=ALU.mult)
                nc.vector.tensor_scalar(zh1[:], z1[:], prm(P_H10), None, op0=ALU.mult)
                nc.vector.tensor_add(p0v, y0, zh0[:].rearrange("p (j t) -> p j t", t=T))
                nc.vector.tensor_add(p0[:], p0[:], zh1[:])
                nc.vector.tensor_scalar(zh0[:], z0[:], prm(P_H01), None, op0=ALU.mult)
                nc.vector.tensor_scalar(zh1[:], z1[:], prm(P_H11), None, op0=ALU.mult)
                nc.vector.tensor_add(p1v, y1, zh0[:].rearrange("p (j t) -> p j t", t=T))
                nc.vector.tensor_add(p1[:], p1[:], zh1[:])

                nc.vector.tensor_mul(lint3[:, :, 0:T], p0v, inv3v)
                nc.vector.tensor_mul(lint3[:, :, T:2 * T], p1v, inv3v)

                prev = (lint3, j0)

            # drain the last supergroup's F phase
            for q in range(NQ):
                emit_F_quad(prev[0], prev[1], q)

    nc.compile()
    return nc


_CACHE = {}


def _get_nc():
    if "nc" not in _CACHE:
        _CACHE["nc"] = build_bass()
    return _CACHE["nc"]


def kernel(**inputs) -> np.ndarray:
    idx = np.asarray(inputs["idx"]).astype(np.int64)
    kw = {k: np.asarray(v, np.float64) for k, v in inputs.items() if k != "idx"}
    R, RHS2, SROW, PRM, CU, SW = host_tables(**kw)
    nc = _get_nc()
    in_maps = []
    for c in range(NCORES):
        bas = build_basis(idx[c * BC:(c + 1) * BC], CU, SW)
        in_maps.append({"basis": bas, "R": R, "RHS2": RHS2,
                        "SROW": SROW, "PRM": PRM})
    res = run_bass_kernel_spmd(nc, in_maps, core_ids=list(range(NCORES)))
    out = np.concatenate([np.asarray(res.results[c]["out"]).astype(np.float32)
                          for c in range(NCORES)], axis=0)
    return np.ascontiguousarray(out.reshape(B, T, V))


# revision 10
# speedup vs baseline: 2.5833x; 1.0034x over previous
"""Trainium2 Bass kernel for nn_MicroAdder (tiny dense transformer).

Decomposition (v3):
  Every per-element quantity in the reference network is affine in the basis
  [u_s, w_s, 1] with u = cos(tok_angle), w = sin(tok_angle) and
  position-dependent constant coefficients.  The HOST gathers u/w from a
  14-entry table and ships the basis directly as fp16 [69, B] -- no on-device
  trig, no int->float casts, and the basis is already transposed so mm1 needs
  no PE transpose.  One PE matmul per 128-row block (lhsT = basis slice,
  rhs = R [69, 272]) produces all 8 linear forms (att, rr, q0, q1, e0, e1,
  y0, y1), with the causal-softmax attention mixing folded into R.  A short
  fp16 elementwise chain (DVE 2x modes; Ln/Exp for the two rsqrt's -- the DVE
  reciprocal is ~8 cycles/elem, Ln+Exp on ACT is ~3x cheaper) produces the two
  logit forms L0, L1.  These are DMA-transposed (XBAR) and expanded to the
  (T,V) logits by a second PE matmul with a block-diagonal constant rhs, then
  stored as bf16 (host converts to fp32).

Sharding: pure data parallel over the batch dim across 8 NeuronCores.
"""

import math
import sys

import numpy as np

for _p in ("/opt/trn_rl_repo", "/root/.axon_site/_ro/trn_rl_repo"):
    if _p not in sys.path:
        sys.path.append(_p)

import concourse.bacc as bacc  # noqa: E402
import concourse.bass as bass  # noqa: E402
import concourse.tile as tile  # noqa: E402
from concourse import mybir  # noqa: E402
from concourse.bass_utils import run_bass_kernel_spmd  # noqa: E402
from concourse.masks import make_identity  # noqa: E402

# ---------------------------------------------------------------- problem dims
B, T, V = 65536, 34, 14
D, EPS, MAX_DIGITS = 5, 1e-5, 10
NCORES = 8
BC = B // NCORES            # rows per core = 8192
P = 128                     # partitions
NBLK = BC // P              # 64 blocks of 128 rows per core
SGB = 16                    # blocks per supergroup
NSG = NBLK // SGB           # 4 supergroups
NQ = SGB // 4               # quads per supergroup = 4
K1 = 2 * T + 1              # basis size = 69
NG = 8                      # mm1 form groups
N1 = NG * T                 # 272
N2 = T * V                  # 476
FW = SGB * T                # 544 chain columns per supergroup
NPRM = 9

F32 = mybir.dt.float32
F16 = mybir.dt.float16
BF16 = mybir.dt.bfloat16
I32 = mybir.dt.int32
AF = mybir.ActivationFunctionType
ALU = mybir.AluOpType

# mm1 output group order (columns g*T..(g+1)*T of R).  ATT/Y0/Y1 adjacent so
# one ACT copy drains all three; the att*rr product then has only one PSUM
# operand (HW allows at most one non-scalar PSUM input per vector op).
G_ATT, G_Y0, G_Y1, G_RR, G_Q0, G_Q1, G_E0, G_E1 = range(8)

# PRM slots
P_RAT, P_SQ0, P_C3, P_H00, P_H10, P_H01, P_H11, P_EPS, P_ZERO = range(9)


# ---------------------------------------------------------------- host tables
def host_tables(tok_A, tok_start, tok_stride, sp_amp, sp_phase, sp_slope, sp_offset,
                norm_w, q_w, q_phase, out_A, out_B, fc1_w, fc2_w, head_w):
    f = np.float64
    A = f(tok_A)
    t = np.arange(T, dtype=f)
    th = 2.0 * np.pi * t / MAX_DIGITS + f(sp_phase)
    pos = np.stack([f(sp_amp) * np.cos(th), f(sp_amp) * np.sin(th),
                    f(sp_slope) * t + f(sp_offset)], axis=-1)
    k = pos @ np.asarray(q_w, f).T
    c0, s0 = np.cos(f(q_phase[0])), np.sin(f(q_phase[0]))
    q = k.copy()
    q[:, 0] = c0 * k[:, 0] - s0 * k[:, 1]
    q[:, 1] = s0 * k[:, 0] + c0 * k[:, 1]
    scores = (q @ k.T) / np.sqrt(f(5.0))
    sm = np.where(np.tril(np.ones((T, T), bool)), scores, -np.inf)
    sm = sm - sm.max(-1, keepdims=True)
    e = np.exp(sm)
    attn = e / e.sum(-1, keepdims=True)

    nw = np.asarray(norm_w, f)
    oA = np.asarray(out_A, f)[:, 0]
    oB = np.asarray(out_B, f)[0]
    S_t = A * A + (pos ** 2).sum(-1)          # |x|^2 per position (tok part = A^2)
    rms1 = np.sqrt(S_t / D + EPS)

    M0 = attn * (A * nw[0] * oA[0] / rms1)[None, :]
    M1 = attn * (A * nw[1] * oA[1] / rms1)[None, :]
    c_t = attn @ ((pos * (nw[2:] * oA[2:])[None, :]).sum(-1) / rms1)

    g0 = np.asarray(fc2_w, f)[:, 0]
    g1 = np.asarray(fc2_w, f)[:, 1]
    projs = {
        G_Q0: nw * np.asarray(fc1_w, f)[0],
        G_Q1: nw * np.asarray(fc1_w, f)[1],
        G_E0: 2.0 * g0,
        G_E1: 2.0 * g1,
        G_Y0: nw * np.asarray(head_w, f)[0],
        G_Y1: nw * np.asarray(head_w, f)[1],
    }
    # R columns: group g covers cols g*T..(g+1)*T; rows: u_s (0:T), w_s (T:2T),
    # const (2T).  att-form = z (attention scalar), rr-form = 2 x.oB + |oB|^2 z.
    R = np.zeros((K1, NG * T), dtype=f)
    dd = np.eye(T, dtype=f)
    b2 = (oB ** 2).sum()
    for gi in range(NG):
        cols = slice(gi * T, (gi + 1) * T)
        if gi == G_ATT:
            R[0:T, cols] = M0.T
            R[T:2 * T, cols] = M1.T
            R[2 * T, cols] = c_t
        elif gi == G_RR:
            R[0:T, cols] = 2 * A * oB[0] * dd + b2 * M0.T
            R[T:2 * T, cols] = 2 * A * oB[1] * dd + b2 * M1.T
            R[2 * T, cols] = 2 * (pos * oB[None, 2:]).sum(-1) + b2 * c_t
        else:
            v = projs[gi]
            bv = (oB * v).sum()
            R[0:T, cols] = A * v[0] * dd + bv * M0.T
            R[T:2 * T, cols] = A * v[1] * dd + bv * M1.T
            R[2 * T, cols] = (pos * v[None, 2:]).sum(-1) + bv * c_t

    G00, G01, G11 = (g0 * g0).sum(), (g0 * g1).sum(), (g1 * g1).sum()
    if G00 > 1e-30:
        sq0, rat = np.sqrt(G00), G01 / G00
        c3 = np.sqrt(max(G11 - G01 * G01 / G00, 0.0))
    else:
        sq0, rat, c3 = 0.0, 0.0, np.sqrt(G11)
    hv0 = nw * np.asarray(head_w, f)[0]
    hv1 = nw * np.asarray(head_w, f)[1]
    H = np.array([[(g0 * hv0).sum(), (g0 * hv1).sum()],
                  [(g1 * hv0).sum(), (g1 * hv1).sum()]])

    dvoc = np.arange(V, dtype=f)
    ang = f(tok_start) + dvoc * f(tok_stride)
    E = np.stack([A * np.cos(ang), A * np.sin(ang)], axis=-1)   # (V, 2)
    RHS2 = np.zeros((2 * T, N2), dtype=f)
    for t_ in range(T):
        RHS2[t_, t_ * V:(t_ + 1) * V] = E[:, 0]
        RHS2[T + t_, t_ * V:(t_ + 1) * V] = E[:, 1]

    # token basis tables: u = cos(ang_v), w = sin(ang_v)
    CU = np.cos(ang)
    SW = np.sin(ang)

    SROW = np.tile(S_t, SGB)[None, :]  # [1, 544]
    PRM = np.zeros((1, NPRM), dtype=f)
    PRM[0, P_RAT] = rat
    PRM[0, P_SQ0] = sq0
    PRM[0, P_C3] = c3
    PRM[0, P_H00] = H[0, 0]
    PRM[0, P_H10] = H[1, 0]
    PRM[0, P_H01] = H[0, 1]
    PRM[0, P_H11] = H[1, 1]
    PRM[0, P_EPS] = EPS
    PRM[0, P_ZERO] = 0.0
    return (R.astype(np.float16), RHS2.astype(np.float16),
            np.ascontiguousarray(SROW, np.float16).copy(),
            np.ascontiguousarray(PRM, np.float32).copy(),
            CU.astype(np.float16), SW.astype(np.float16))


def build_basis(idx, CU, SW):
    """[69, B] fp16: rows 0:T = u_t(b), rows T:2T = w_t(b), row 2T = 1."""
    bset = np.empty((K1, idx.shape[0]), np.float16)
    bset[0:T, :] = CU[idx].T
    bset[T:2 * T, :] = SW[idx].T
    bset[2 * T, :] = np.float16(1.0)
    return bset


def act_raw(nc, out, in_, func, bias, scale):
    """Emit InstActivation directly (same lowering as BassScalarEngine.
    activation) for funcs the wrapper refuses (Rsqrt)."""
    se = nc.scalar
    inputs = [se.lower_ap(in_)]
    for arg in (bias, scale, 0.0):
        if isinstance(arg, bass.AP):
            inputs.append(se.lower_ap(arg))
        else:
            inputs.append(mybir.ImmediateValue(dtype=mybir.dt.float32, value=arg))
    return se.add_instruction(mybir.InstActivation(
        name=se.bass.get_next_instruction_name(),
        func=func, ins=inputs, outs=[se.lower_ap(out)]))


# ---------------------------------------------------------------- bass kernel
def build_bass(prm_vals):
    nc = bacc.Bacc("TRN2", target_bir_lowering=False, debug=False)

    basis_d = nc.dram_tensor("basis", [K1, BC], F16, kind="ExternalInput").ap()
    r_d = nc.dram_tensor("R", [K1, N1], F16, kind="ExternalInput").ap()
    rhs2_d = nc.dram_tensor("RHS2", [2 * T, N2], F16, kind="ExternalInput").ap()
    srow_d = nc.dram_tensor("SROW", [1, FW], F16, kind="ExternalInput").ap()
    prm_d = nc.dram_tensor("PRM", [1, NPRM], F32, kind="ExternalInput").ap()
    out_d = nc.dram_tensor("out", [BC, N2], BF16, kind="ExternalOutput").ap()

    # DRAM out view: block n covers batch rows n*128..n*128+127, partition p
    # holds row n*128+p (matches mm1 lhsT = basis columns n*128+p)
    out_v = out_d.rearrange("(n p) c -> p n c", p=P)       # [128, 64, 476]

    with tile.TileContext(nc) as tc:
        with (
            tc.tile_pool(name="const", bufs=1) as cpool,
            tc.tile_pool(name="bas", bufs=2) as basp,
            tc.tile_pool(name="dr", bufs=2) as drp,
            tc.tile_pool(name="ch", bufs=2) as chp,
            tc.tile_pool(name="ltT", bufs=4) as ltp,
            tc.tile_pool(name="outsb", bufs=3) as outp,
            tc.tile_pool(name="pmm1", bufs=1, space="PSUM") as pmm1p,
            tc.tile_pool(name="pout", bufs=1, space="PSUM") as poutp,
            tc.tile_pool(name="ptr", bufs=2, space="PSUM") as ptrp,
        ):
            # ---- constants
            ident = cpool.tile([P, P], F16)
            make_identity(nc, ident[:])
            r_sb = cpool.tile([K1, N1], F16)
            nc.sync.dma_start(r_sb[:], r_d)
            rhs2_sb = cpool.tile([2 * T, N2], F16)
            nc.sync.dma_start(rhs2_sb[:], rhs2_d)
            s_sb = cpool.tile([P, FW], F16)
            nc.sync.dma_start(s_sb[:], srow_d.broadcast_to([P, FW]))
            prm_sb = cpool.tile([P, NPRM], F32)
            nc.sync.dma_start(prm_sb[:], prm_d.broadcast_to([P, NPRM]))

            def prm(i):
                return prm_sb[:, i:i + 1]

            def pv(i):
                return float(prm_vals[i])

            # persistent double-buffered L-form tiles; cols 68:128 of each
            # block stay zero (transposed padding rows are never read by mm2,
            # but the XBAR transpose reads them)
            lints = [cpool.tile([P, SGB * P], F16, tag=f"lint{i}", name=f"lint{i}")
                     for i in range(2)]
            for lt in lints:
                nc.vector.memset(lt[:], 0.0)

            # state carried between supergroups for the pipelined F phase
            prev = None  # (lint3 view, j0 of previous supergroup)

            def emit_F_quad(lint3, j0, q):
                """Transpose + mm2 + drain + store for blocks q*4..q*4+3."""
                pt = ptrp.tile([P, 4 * P], F16, tag="pt")
                pt4 = pt[:].rearrange("p (k c) -> p k c", k=4)
                for k in range(4):
                    nc.tensor.transpose(pt4[:, k, :], lint3[:, q * 4 + k, :],
                                        ident[:])
                lt = ltp.tile([P, 4 * P], F16, tag="ltT")
                nc.vector.tensor_copy(lt[:], pt[:])
                lt4 = lt[:].rearrange("p (k c) -> p k c", k=4)
                for h in range(2):
                    po = poutp.tile([P, 2 * 512], F32, tag="po")
                    po2 = po[:].rearrange("p (k c) -> p k c", k=2)
                    for k in range(2):
                        nc.tensor.matmul(po2[:, k, 0:N2],
                                         lt4[0:2 * T, 2 * h + k, :],
                                         rhs2_sb[:], start=True, stop=True)
                    o_sb = outp.tile([P, 2, N2], BF16, tag="osb")
                    eng = (nc.vector, nc.scalar)[(2 * q + h) % 2]
                    if eng is nc.scalar:
                        nc.scalar.copy(o_sb[:], po2[:, :, 0:N2])
                    else:
                        eng.tensor_copy(o_sb[:], po2[:, :, 0:N2])
                    jb = j0 + q * 4 + 2 * h
                    nc.sync.dma_start(out_v[:, jb:jb + 2, :], o_sb[:])

            for sg in range(NSG):
                j0 = sg * SGB
                lint = lints[sg % 2]
                lint3 = lint[:].rearrange("p (j c) -> p j c", c=P)

                # ---------------- phase A: basis DMA  [69, 2048]
                b_t = basp.tile([K1, SGB * P], F16, tag="bas")
                nc.sync.dma_start(b_t[:], basis_d[:, j0 * P:(j0 + SGB) * P])

                # drain targets for the whole supergroup (fp16)
                ayy = drp.tile([P, SGB, 3 * T], F16, tag="ayy")
                ar = drp.tile([P, SGB, T], F16, tag="ar")
                rho = drp.tile([P, SGB, 2 * T], F16, tag="rho")
                tab = drp.tile([P, SGB, 2 * T], F16, tag="tab")

                # ---------------- phase B: mm1 per quad + drains
                for q in range(NQ):
                    pm = pmm1p.tile([P, 4 * 512], F32, tag="mm1")
                    pm4 = pm[:].rearrange("p (k c) -> p k c", k=4)
                    for k in range(4):
                        j = q * 4 + k
                        nc.tensor.matmul(
                            pm4[:, k, 0:N1],
                            b_t[:, j * P:(j + 1) * P],
                            r_sb[:], start=True, stop=True)
                    qs = slice(q * 4, q * 4 + 4)

                    def g(g0, g1=None):
                        g1 = g0 if g1 is None else g1
                        return pm4[:, :, g0 * T:(g1 + 1) * T]

                    nc.scalar.copy(ayy[:, qs, :], g(G_ATT, G_Y1))
                    nc.vector.tensor_mul(ar[:, qs, :], ayy[:, qs, 0:T], g(G_RR))
                    nc.vector.tensor_scalar_max(rho[:, qs, :], g(G_Q0, G_Q1), 0.0)
                    nc.vector.tensor_mul(tab[:, qs, :], rho[:, qs, :], g(G_E0, G_E1))

                    # pipelined phase F of the previous supergroup
                    if prev is not None:
                        emit_F_quad(prev[0], prev[1], q)

                # ---------------- phase D: fp16 chain on [128, 544]
                arf = ar[:].rearrange("p j t -> p (j t)")
                rho0 = rho[:, :, 0:T]
                rho1 = rho[:, :, T:2 * T]
                y0 = ayy[:, :, T:2 * T]
                y1 = ayy[:, :, 2 * T:3 * T]

                def ct(tag):
                    t_ = chp.tile([P, FW], F16, tag=tag)
                    return t_, t_[:].rearrange("p (j t) -> p j t", t=T)

                n2, n2v = ct("n2")
                nc.gpsimd.tensor_add(n2[:], arf, s_sb[:])
                inv2, inv2v = ct("inv2")
                act_raw(nc, inv2[:], n2[:], AF.Rsqrt, prm(P_EPS), 1.0 / D)

                z0, z0v = ct("z0")
                nc.gpsimd.tensor_mul(z0v, rho0, inv2v)
                z1, z1v = ct("z1")
                nc.gpsimd.tensor_mul(z1v, rho1, inv2v)
                tau, tauv = ct("tau")
                nc.gpsimd.tensor_add(tauv, tab[:, :, 0:T], tab[:, :, T:2 * T])
                it2, it2v = ct("it2")
                nc.gpsimd.tensor_mul(it2[:], tau[:], inv2[:])

                v1, _ = ct("v1")
                nc.vector.tensor_scalar(v1[:], z1[:], pv(P_RAT), None, op0=ALU.mult)
                nc.vector.tensor_add(v1[:], v1[:], z0[:])
                q1s, _ = ct("q1s")
                nc.scalar.activation(q1s[:], v1[:], AF.Square, bias=prm(P_ZERO),
                                     scale=pv(P_SQ0))
                q2s, _ = ct("q2s")
                nc.scalar.activation(q2s[:], z1[:], AF.Square, bias=prm(P_ZERO),
                                     scale=pv(P_C3))

                n3, _ = ct("n3")
                nc.gpsimd.tensor_add(n3[:], n2[:], it2[:])
                nc.gpsimd.tensor_add(n3[:], n3[:], q1s[:])
                nc.gpsimd.tensor_add(n3[:], n3[:], q2s[:])
                inv3, inv3v = ct("inv3")
                act_raw(nc, inv3[:], n3[:], AF.Rsqrt, prm(P_EPS), 1.0 / D)

                zh0, _ = ct("zh0")
                zh1, _ = ct("zh1")
                p0, p0v = ct("p0")
                p1, p1v = ct("p1")
                nc.vector.tensor_scalar(zh0[:], z0[:], pv(P_H00), None, op0=ALU.mult)
                nc.vector.tensor_scalar(zh1[:], z1[:], pv(P_H10), None, op0=ALU.mult)
                nc.vector.tensor_add(p0v, y0, zh0[:].rearrange("p (j t) -> p j t", t=T))
                nc.vector.tensor_add(p0[:], p0[:], zh1[:])
                nc.vector.tensor_scalar(zh0[:], z0[:], pv(P_H01), None, op0=ALU.mult)
                nc.vector.tensor_scalar(zh1[:], z1[:], pv(P_H11), None, op0=ALU.mult)
                nc.vector.tensor_add(p1v, y1, zh0[:].rearrange("p (j t) -> p j t", t=T))
                nc.vector.tensor_add(p1[:], p1[:], zh1[:])

                nc.vector.tensor_mul(lint3[:, :, 0:T], p0v, inv3v)
                nc.vector.tensor_mul(lint3[:, :, T:2 * T], p1v, inv3v)

                prev = (lint3, j0)

            # drain the last supergroup's F phase
            for q in range(NQ):
                emit_F_quad(prev[0], prev[1], q)

    nc.compile()
    return nc


_CACHE = {}


def _get_nc(PRM):
    key = PRM.tobytes()
    if _CACHE.get("key") != key:
        _CACHE["nc"] = build_bass(PRM[0])
        _CACHE["key"] = key
    return _CACHE["nc"]


def kernel(**inputs) -> np.ndarray:
    idx = np.asarray(inputs["idx"]).astype(np.int64)
    kw = {k: np.asarray(v, np.float64) for k, v in inputs.items() if k != "idx"}
    R, RHS2, SROW, PRM, CU, SW = host_tables(**kw)
    nc = _get_nc(PRM)
    in_maps = []
    for c in range(NCORES):
        bas = build_basis(idx[c * BC:(c + 1) * BC], CU, SW)
        in_maps.append({"basis": bas, "R": R, "RHS2": RHS2,
                        "SROW": SROW, "PRM": PRM})
    res = run_bass_kernel_spmd(nc, in_maps, core_ids=list(range(NCORES)))
    out = np.concatenate([np.asarray(res.results[c]["out"]).astype(np.float32)
                          for c in range(NCORES)], axis=0)
    return np.ascontiguousarray(out.reshape(B, T, V))
